# revision 2
# baseline (speedup 1.0000x reference)
"""Trainium2 Bass kernel for a pre-norm transformer block (attention + MLP).

Sharding: batch (2) x query-block (4) across 8 cores. Each core computes
LN1 + K/V over its full batch (replicated within its 4-core group) and
attention / projection / MLP for its own 1024 query tokens. No collectives.

Device layouts (per core):
  xT  : LN1(x) feature-major [128c, 4ct, 4096t] bf16
  kT  : per head-pair hp [128 (2 heads x 64d), 4096m] bf16
  v   : token-major [128m, 32mt, 8h, 65] bf16 (65th col = ones -> softmax denom)
  S^T : [128m, 2h, 512n] PSUM (keys on partitions; head pair row-packed)
  A@V : out^T[65, n]: lhsT=[V|1] per head, accumulated over 32 m-tiles
"""

import numpy as np
import ml_dtypes

B, N, C = 2, 4096, 512
H, D = 8, 64
HID = 2048
NQ = 1024
NCORES = 8
EPS = 1e-5
BF = ml_dtypes.bfloat16

_CACHE = {}


def _build_program(repeat=1):
    from concourse import bacc
    import concourse.bass as bass
    import concourse.mybir as mybir
    from concourse.tile import TileContext

    dt = mybir.dt
    AF = mybir.ActivationFunctionType
    ALU = mybir.AluOpType

    nc = bacc.Bacc(None, target_bir_lowering=False)

    xfull = nc.dram_tensor("xfull", (N, C), dt.float32, kind="ExternalInput")
    xq = nc.dram_tensor("xq", (NQ, C), dt.float32, kind="ExternalInput")
    wq_d = nc.dram_tensor("wq_d", (128, 4, C), dt.bfloat16, kind="ExternalInput")
    wk_d = nc.dram_tensor("wk_d", (128, 4, C), dt.bfloat16, kind="ExternalInput")
    wv_d = nc.dram_tensor("wv_d", (128, 4, C), dt.bfloat16, kind="ExternalInput")
    wp_d = nc.dram_tensor("wp_d", (64, 8, C), dt.bfloat16, kind="ExternalInput")
    w1_d = nc.dram_tensor("w1_d", (128, 4, HID), dt.bfloat16, kind="ExternalInput")
    w2_d = nc.dram_tensor("w2_d", (128, 16, C), dt.bfloat16, kind="ExternalInput")
    bq_d = nc.dram_tensor("bq_d", (128, 4), dt.float32, kind="ExternalInput")
    bk_d = nc.dram_tensor("bk_d", (128, 4), dt.float32, kind="ExternalInput")
    bv_d = nc.dram_tensor("bv_d", (C,), dt.float32, kind="ExternalInput")
    bp_d = nc.dram_tensor("bp_d", (C,), dt.float32, kind="ExternalInput")
    b1_d = nc.dram_tensor("b1_d", (128, 16), dt.float32, kind="ExternalInput")
    b2_d = nc.dram_tensor("b2_d", (C,), dt.float32, kind="ExternalInput")
    y = nc.dram_tensor("y", (NQ, C), dt.float32, kind="ExternalOutput")

    xfull_t = xfull.rearrange("(i p) c -> p i c", p=128)
    xq_t = xq.rearrange("(i p) c -> p i c", p=128)
    y_t = y.rearrange("(i p) c -> p i c", p=128)

    with TileContext(nc) as tc:
      for rep in range(repeat):
        R = f"r{rep}_"
        with tc.tile_pool(name=R + "persist", bufs=1) as pers, \
             tc.tile_pool(name=R + "stat", bufs=4) as statp, \
             tc.tile_pool(name=R + "stream", bufs=3) as stream:

            eps_t = pers.tile([128, 1], dt.float32, name=R + "eps")
            nc.vector.memset(eps_t, EPS)
            ones_sb = pers.tile([128, 64], dt.float32, name=R + "ones")
            nc.vector.memset(ones_sb, 1.0)

            xq_sb = pers.tile([128, 8, C], dt.float32, name=R + "xq_sb")
            nc.sync.dma_start(out=xq_sb, in_=xq_t[:])

            def ln_to_T(src_sb_or_dram, i, xT, tag, from_sbuf=False):
                """One 128-token LN tile -> standardized bf16 -> transposed into xT."""
                if from_sbuf:
                    xt = src_sb_or_dram
                else:
                    xt = stream.tile([128, C], dt.float32, tag=tag + "x", name=f"{R}{tag}x{i}")
                    nc.sync.dma_start(out=xt, in_=src_sb_or_dram)
                stats = statp.tile([128, 6], dt.float32, tag=tag + "st", name=f"{R}{tag}st{i}")
                mv = statp.tile([128, 2], dt.float32, tag=tag + "mv", name=f"{R}{tag}mv{i}")
                nc.vector.bn_stats(stats, xt)
                nc.vector.bn_aggr(mv, stats)
                sq = statp.tile([128, 1], dt.float32, tag=tag + "sq", name=f"{R}{tag}sq{i}")
                nc.scalar.activation(sq, mv[:, 1:2], AF.Sqrt, bias=eps_t)
                rstd = statp.tile([128, 1], dt.float32, tag=tag + "rs", name=f"{R}{tag}rs{i}")
                nc.vector.reciprocal(rstd, sq)
                xn = stream.tile([128, C], dt.bfloat16, tag=tag + "n", name=f"{R}{tag}n{i}")
                nc.vector.tensor_scalar(xn, xt, mv[:, 0:1], rstd, ALU.subtract, ALU.mult)
                for co in range(4):
                    nc.sync.dma_start(out=xT[:, co, i * 128:(i + 1) * 128],
                                      in_=xn[:, co * 128:(co + 1) * 128], transpose=True)

            # ---------------- attention phase ----------------
            with tc.tile_pool(name=R + "attw", bufs=1) as attw, \
                 tc.tile_pool(name=R + "attbig", bufs=1) as attbig, \
                 tc.tile_pool(name=R + "kq", bufs=2) as kqp:
                wq_sb = attw.tile([128, 4, C], dt.bfloat16, name=R + "wq_sb")
                wk_sb = attw.tile([128, 4, C], dt.bfloat16, name=R + "wk_sb")
                wv_sb = attw.tile([128, 4, C], dt.bfloat16, name=R + "wv_sb")
                wp_sb = attw.tile([64, 8, C], dt.bfloat16, name=R + "wp_sb")
                bq_sb = attw.tile([128, 4], dt.float32, name=R + "bq_sb")
                bk_sb = attw.tile([128, 4], dt.float32, name=R + "bk_sb")
                bv_sb = attw.tile([128, 8, 64], dt.bfloat16, name=R + "bv_sb")
                bp_sb = attw.tile([128, C], dt.float32, name=R + "bp_sb")
                nc.sync.dma_start(out=wq_sb, in_=wq_d[:])
                nc.sync.dma_start(out=wk_sb, in_=wk_d[:])
                nc.sync.dma_start(out=wv_sb, in_=wv_d[:])
                nc.sync.dma_start(out=wp_sb, in_=wp_d[:])
                nc.sync.dma_start(out=bq_sb, in_=bq_d[:])
                nc.sync.dma_start(out=bk_sb, in_=bk_d[:])
                nc.gpsimd.dma_start(out=bv_sb, in_=bass.AP(tensor=bv_d, offset=0, ap=[[0, 128], [1, C]]))
                nc.gpsimd.dma_start(out=bp_sb, in_=bass.AP(tensor=bp_d, offset=0, ap=[[0, 128], [1, C]]))

                xT = attbig.tile([128, 4, N], dt.bfloat16, name=R + "xT")
                xqT = attbig.tile([128, 4, NQ], dt.bfloat16, name=R + "xqT")
                for i in range(32):
                    ln_to_T(xfull_t[:, i, :], i, xT, "l1")
                for i in range(8):
                    ln_to_T(xq_t[:, i, :], i, xqT, "lq")

                v_sb = attbig.tile([128, 32, H, 65], dt.bfloat16, name=R + "v_sb")
                nc.vector.memset(v_sb[:, :, :, 64:65], 1.0)
                with tc.tile_pool(name=R + "pv", bufs=4, space="PSUM") as pv:
                    for mt in range(32):
                        ps_v = pv.tile([128, C], dt.float32, tag="ps_v", name=f"{R}ps_v{mt}")
                        for kt in range(4):
                            nc.tensor.matmul(ps_v, xT[:, kt, mt * 128:(mt + 1) * 128],
                                             wv_sb[:, kt, :], start=(kt == 0), stop=(kt == 3))
                        nc.vector.tensor_tensor(
                            out=v_sb[:, mt, :, 0:64],
                            in0=ps_v.rearrange("p (h d) -> p h d", h=H),
                            in1=bv_sb, op=ALU.add)

                ao_lo = attbig.tile([64, 4, NQ], dt.bfloat16, name=R + "ao_lo")
                ao_hi = attbig.tile([64, 4, NQ], dt.bfloat16, name=R + "ao_hi")

                for hp in range(4):
                    kT = kqp.tile([128, N], dt.bfloat16, tag="kT", name=f"{R}kT{hp}")
                    qT = kqp.tile([128, NQ], dt.bfloat16, tag="qT", name=f"{R}qT{hp}")
                    with tc.tile_pool(name=f"{R}pk{hp}", bufs=4, space="PSUM") as pk:
                        for nch in range(8):
                            ps_k = pk.tile([128, 512], dt.float32, tag="ps_k", name=f"{R}ps_k{hp}_{nch}")
                            for kt in range(4):
                                nc.tensor.matmul(ps_k, wk_sb[:, kt, hp * 128:(hp + 1) * 128],
                                                 xT[:, kt, nch * 512:(nch + 1) * 512],
                                                 start=(kt == 0), stop=(kt == 3))
                            nc.vector.tensor_scalar(kT[:, nch * 512:(nch + 1) * 512],
                                                    ps_k, bk_sb[:, hp:hp + 1], None, ALU.add)
                        for nch in range(2):
                            ps_q = pk.tile([128, 512], dt.float32, tag="ps_k", name=f"{R}ps_q{hp}_{nch}")
                            for kt in range(4):
                                nc.tensor.matmul(ps_q, wq_sb[:, kt, hp * 128:(hp + 1) * 128],
                                                 xqT[:, kt, nch * 512:(nch + 1) * 512],
                                                 start=(kt == 0), stop=(kt == 3))
                            nc.vector.tensor_scalar(qT[:, nch * 512:(nch + 1) * 512],
                                                    ps_q, bq_sb[:, hp:hp + 1], None, ALU.add)

                    with tc.tile_pool(name=f"{R}pa{hp}", bufs=1, space="PSUM") as pa, \
                         tc.tile_pool(name=f"{R}ptp{hp}", bufs=3) as ptp:
                        for nch in range(2):
                            nsl = slice(nch * 512, (nch + 1) * 512)
                            po = [pa.tile([65, 512], dt.float32, tag=f"po{h}", name=f"{R}po{hp}_{nch}_{h}")
                                  for h in range(2)]
                            for mt in range(32):
                                msl = slice(mt * 128, (mt + 1) * 128)
                                ps_s = pa.tile([128, 2, 512], dt.float32, bufs=3,
                                               tag="ps_s", name=f"{R}ps_s{hp}_{nch}_{mt}")
                                nc.tensor.matmul(ps_s[:, 0, :], kT[0:64, msl], qT[0:64, nsl],
                                                 start=True, stop=True)
                                nc.tensor.matmul(ps_s[:, 1, :], kT[64:128, msl], qT[64:128, nsl],
                                                 start=True, stop=True, tile_position=(64, 0))
                                pt = ptp.tile([128, 2, 512], dt.bfloat16, tag="pt",
                                              name=f"{R}pt{hp}_{nch}_{mt}")
                                nc.scalar.activation(pt, ps_s, AF.Exp, scale=float(D) ** -0.5)
                                for h in range(2):
                                    nc.tensor.matmul(po[h], v_sb[:, mt, 2 * hp + h, :],
                                                     pt[:, h, :], start=(mt == 0), stop=(mt == 31))
                            for h in range(2):
                                ao_dst = ao_lo if h == 0 else ao_hi
                                rden = statp.tile([128, 512], dt.float32, bufs=2,
                                                  tag="rden", name=f"{R}rden{hp}_{nch}_{h}")
                                nc.vector.reciprocal(rden[64:65, :], po[h][64:65, :])
                                bc_ps = pa.tile([64, 512], dt.float32, bufs=3,
                                                tag="ps_s", name=f"{R}bc{hp}_{nch}_{h}")
                                nc.tensor.matmul(bc_ps, ones_sb[64:65, :], rden[64:65, :],
                                                 start=True, stop=True, tile_position=(64, 0))
                                bc_sb = statp.tile([64, 512], dt.float32, bufs=2,
                                                   tag="bcs", name=f"{R}bcs{hp}_{nch}_{h}")
                                nc.vector.tensor_copy(bc_sb, bc_ps)
                                nc.vector.tensor_tensor(out=ao_dst[:, hp, nsl],
                                                        in0=po[h][0:64, :], in1=bc_sb, op=ALU.mult)

                with tc.tile_pool(name=R + "pw", bufs=4, space="PSUM") as pw:
                    for ns in range(8):
                        ps_p = pw.tile([128, C], dt.float32, tag="ps_p", name=f"{R}ps_p{ns}")
                        qsl = slice(ns * 128, (ns + 1) * 128)
                        for hp in range(4):
                            nc.tensor.matmul(ps_p, ao_lo[:, hp, qsl], wp_sb[:, 2 * hp, :],
                                             start=(hp == 0), stop=False)
                            nc.tensor.matmul(ps_p, ao_hi[:, hp, qsl], wp_sb[:, 2 * hp + 1, :],
                                             start=False, stop=(hp == 3))
                        nc.vector.tensor_tensor(out=xq_sb[:, ns, :], in0=xq_sb[:, ns, :],
                                                in1=ps_p, op=ALU.add)
                        nc.vector.tensor_tensor(out=xq_sb[:, ns, :], in0=xq_sb[:, ns, :],
                                                in1=bp_sb, op=ALU.add)

            # ---------------- MLP phase ----------------
            with tc.tile_pool(name=R + "mlpw", bufs=1) as mlpw, \
                 tc.tile_pool(name=R + "mlpbig", bufs=1) as mlpbig:
                w1_sb = mlpw.tile([128, 4, HID], dt.bfloat16, name=R + "w1_sb")
                w2_sb = mlpw.tile([128, 16, C], dt.bfloat16, name=R + "w2_sb")
                b1_sb = mlpw.tile([128, 16], dt.float32, name=R + "b1_sb")
                b2_sb = mlpw.tile([128, C], dt.float32, name=R + "b2_sb")
                nc.sync.dma_start(out=w1_sb, in_=w1_d[:])
                nc.sync.dma_start(out=w2_sb, in_=w2_d[:])
                nc.sync.dma_start(out=b1_sb, in_=b1_d[:])
                nc.gpsimd.dma_start(out=b2_sb, in_=bass.AP(tensor=b2_d, offset=0, ap=[[0, 128], [1, C]]))

                x2T = mlpbig.tile([128, 4, NQ], dt.bfloat16, name=R + "x2T")
                for i in range(8):
                    ln_to_T(xq_sb[:, i, :], i, x2T, "l2", from_sbuf=True)

                h_sb = mlpbig.tile([128, 16, NQ], dt.bfloat16, name=R + "h_sb")
                with tc.tile_pool(name=R + "ph", bufs=4, space="PSUM") as ph:
                    for pt_i in range(16):
                        for nch in range(2):
                            ps_h = ph.tile([128, 512], dt.float32, tag="ps_h",
                                           name=f"{R}ps_h{pt_i}_{nch}")
                            for kt in range(4):
                                nc.tensor.matmul(ps_h, w1_sb[:, kt, pt_i * 128:(pt_i + 1) * 128],
                                                 x2T[:, kt, nch * 512:(nch + 1) * 512],
                                                 start=(kt == 0), stop=(kt == 3))
                            nc.vector.tensor_scalar(h_sb[:, pt_i, nch * 512:(nch + 1) * 512],
                                                    ps_h, b1_sb[:, pt_i:pt_i + 1], 0.0,
                                                    ALU.add, ALU.max)
                    for ns in range(8):
                        ps_m = ph.tile([128, C], dt.float32, tag="ps_m", name=f"{R}ps_m{ns}")
                        qsl = slice(ns * 128, (ns + 1) * 128)
                        for kt in range(16):
                            nc.tensor.matmul(ps_m, h_sb[:, kt, qsl], w2_sb[:, kt, :],
                                             start=(kt == 0), stop=(kt == 15))
                        ot = stream.tile([128, C], dt.float32, tag="out", name=f"{R}out{ns}")
                        nc.vector.tensor_tensor(out=ot, in0=ps_m, in1=xq_sb[:, ns, :], op=ALU.add)
                        nc.vector.tensor_tensor(out=ot, in0=ot, in1=b2_sb, op=ALU.add)
                        nc.sync.dma_start(out=y_t[:, ns, :], in_=ot)

    nc.finalize()
    return nc


def _prepare_host(inputs):
    f32 = np.float32
    x = np.asarray(inputs["x"], f32)
    ln1_w = np.asarray(inputs["ln1_w"], f32); ln1_b = np.asarray(inputs["ln1_b"], f32)
    ln2_w = np.asarray(inputs["ln2_w"], f32); ln2_b = np.asarray(inputs["ln2_b"], f32)
    wq = np.asarray(inputs["wq"], f32); wkv = np.asarray(inputs["wkv"], f32)
    wp = np.asarray(inputs["wp"], f32); bp = np.asarray(inputs["bp"], f32)
    w1 = np.asarray(inputs["w1"], f32); b1 = np.asarray(inputs["b1"], f32)
    w2 = np.asarray(inputs["w2"], f32); b2 = np.asarray(inputs["b2"], f32)

    wq_f = ln1_w[:, None] * wq
    wkv_f = ln1_w[:, None] * wkv
    w1_f = ln2_w[:, None] * w1
    bq_f = ln1_b @ wq
    bkv_f = ln1_b @ wkv
    b1_f = b1 + ln2_b @ w1

    def kmaj(w, cols, kt):
        return np.ascontiguousarray(w.reshape(kt, 128, cols).transpose(1, 0, 2)).astype(BF)

    shared = dict(
        wq_d=kmaj(wq_f, C, 4),
        wk_d=kmaj(wkv_f[:, :C], C, 4),
        wv_d=kmaj(wkv_f[:, C:], C, 4),
        wp_d=np.ascontiguousarray(wp.reshape(H, D, C).transpose(1, 0, 2)).astype(BF),
        w1_d=kmaj(w1_f, HID, 4),
        w2_d=np.ascontiguousarray(w2.reshape(16, 128, C).transpose(1, 0, 2)).astype(BF),
        bq_d=np.ascontiguousarray(bq_f.reshape(4, 128).T).astype(f32),
        bk_d=np.ascontiguousarray(bkv_f[:C].reshape(4, 128).T).astype(f32),
        bv_d=np.ascontiguousarray(bkv_f[C:]).astype(f32),
        bp_d=np.ascontiguousarray(bp).astype(f32),
        b1_d=np.ascontiguousarray(b1_f.reshape(16, 128).T).astype(f32),
        b2_d=np.ascontiguousarray(b2).astype(f32),
    )

    in_maps = []
    for core in range(NCORES):
        bi, qi = divmod(core, 4)
        in_maps.append(dict(shared,
                            xfull=np.ascontiguousarray(x[bi]),
                            xq=np.ascontiguousarray(x[bi, qi * NQ:(qi + 1) * NQ])))
    return in_maps


def kernel(**inputs):
    from concourse.bass_utils import run_bass_kernel_spmd

    if "nc" not in _CACHE:
        _CACHE["nc"] = _build_program()
    nc = _CACHE["nc"]
    in_maps = _prepare_host(inputs)
    res = run_bass_kernel_spmd(nc, in_maps, core_ids=list(range(NCORES)))
    out = np.empty((B, N, C), np.float32)
    for core in range(NCORES):
        bi, qi = divmod(core, 4)
        out[bi, qi * NQ:(qi + 1) * NQ] = res.results[core]["y"]
    return out
